# revision 17
# baseline (speedup 1.0000x reference)
"""AnatomyGAT (2-layer RGAT over 1024 graphs) on 8 TRN2 NeuronCores, Bass/Tile.

Sharding: node-parallel. Core c owns nodes [c*6144,(c+1)*6144); edges live on
the dst-owner core, grouped per (dst slot of 128 nodes, relation, src-half)
into 128-edge chunks with a chunk structure that is the max over cores (SPMD
static program; pads use dummy src index 0 and zero rows in the one-hot M).

Per chunk: transpose dma_gather of h[src] (bf16) -> TensorE per-edge
transform [oj|kj] = h_src @ [W_r|W_r k] -> w = exp(lrelu(qi[dst]+kj)) with
qi[dst] expanded by M^T matmul -> U += M^T @ [w*oj | w] in PSUM per slot.
Segment softmax denominator folded in at node level: U/(S+1e-16).
h is AllGathered (bf16) once per layer; per-graph LN stats via one-hot
matmuls + AllReduce + stats-table gather by batch id.
"""

import numpy as np
import ml_dtypes

import concourse.bass as bass
import concourse.bacc as bacc
import concourse.mybir as mybir
import concourse.tile as tile
from concourse.bass_utils import run_bass_kernel_spmd

BF16 = ml_dtypes.bfloat16
F32 = mybir.dt.float32
BF = mybir.dt.bfloat16
I16 = mybir.dt.int16

N, G, R, H, C, F = 49152, 1024, 3, 8, 48, 384
NCORES = 8
NS = N // NCORES          # 6144
NSLOT = NS // 128         # 48
NBATCH = 8
BS = NSLOT // NBATCH      # 6
SPLIT = 32767
NEG = 0.2
EPS = 1e-5
AF = mybir.ActivationFunctionType
ALU = mybir.AluOpType


def _wrap_idx(idx):
    idx = np.asarray(idx, np.int16)
    assert len(idx) % 16 == 0
    return np.tile(idx.reshape(-1, 16).T, (8, 1))


def preprocess(inp):
    f32 = np.float32
    d = {"shared": {}, "percore": [dict() for _ in range(NCORES)]}
    sh = d["shared"]

    # ---- weights ----
    for l, pfx in ((0, "r1"), (1, "r2")):
        W = np.asarray(inp[f"{pfx}_w"], f32)              # [R,384,384]
        q = np.asarray(inp[f"{pfx}_q"], f32)              # [384,8]
        k = np.asarray(inp[f"{pfx}_k"], f32)
        waug = np.concatenate([W, W @ k], axis=2)         # [R,384,392]
        # store [128, kchunk(3), r(3), 392]
        sh[f"waug{l}"] = np.ascontiguousarray(
            waug.reshape(R, 3, 128, 392).transpose(2, 1, 0, 3)
            .reshape(128, 3 * R * 392)).astype(BF16)
        wq = W @ q                                        # [R,384,8]
        sh[f"wq{l}"] = np.ascontiguousarray(
            wq.reshape(R, 3, 128, 8).transpose(2, 1, 0, 3)
            .reshape(128, 3 * R * 8)).astype(BF16)
        sh[f"rb{l}"] = np.repeat(np.asarray(inp[f"{pfx}_b"], f32).reshape(1, F), 128, 0)
        sh[f"nw{l}"] = np.repeat(np.asarray(inp[f"n{l+1}_w"], f32).reshape(1, F), 128, 0)
        sh[f"nb{l}"] = np.repeat(np.asarray(inp[f"n{l+1}_b"], f32).reshape(1, F), 128, 0)

    vis_w = np.asarray(inp["vis_w"], f32)                 # [1024,128]
    sh["visw"] = np.ascontiguousarray(
        vis_w.reshape(8, 128, 128).transpose(1, 0, 2).reshape(128, 8 * 128)).astype(BF16)
    gw = np.zeros((8, 128), f32); gw[:6] = np.asarray(inp["geom_w"], f32)
    sh["gw"] = gw.astype(BF16)
    pw = np.zeros((64, 128), f32); pw[:50] = np.asarray(inp["prior_w"], f32)
    sh["pw"] = pw.astype(BF16)
    sh["encb"] = np.repeat(np.concatenate([np.asarray(inp["vis_b"], f32),
                                 np.asarray(inp["geom_b"], f32),
                                 np.asarray(inp["prior_b"], f32)]).reshape(1, F), 128, 0)
    sh["enclw"] = np.repeat(np.concatenate([np.asarray(inp["vis_lw"], f32),
                                  np.asarray(inp["geom_lw"], f32),
                                  np.asarray(inp["prior_lw"], f32)]).reshape(1, F), 128, 0)
    sh["enclb"] = np.repeat(np.concatenate([np.asarray(inp["vis_lb"], f32),
                                  np.asarray(inp["geom_lb"], f32),
                                  np.asarray(inp["prior_lb"], f32)]).reshape(1, F), 128, 0)
    cw1 = np.asarray(inp["c_w1"], f32)                    # [384,128]
    sh["cw1"] = np.ascontiguousarray(
        cw1.reshape(3, 128, 128).transpose(1, 0, 2).reshape(128, 3 * 128)).astype(BF16)
    sh["cb1"] = np.asarray(inp["c_b1"], f32).reshape(128, 1)
    sh["cw2"] = np.asarray(inp["c_w2"], f32).astype(BF16)
    sh["cb2"] = np.repeat(np.asarray(inp["c_b2"], f32).reshape(1, 49), 128, 0)

    # ---- feature shards (transposed, bf16) ----
    xv = np.asarray(inp["x_visual"], f32)
    xg = np.zeros((N, 8), f32); xg[:, :6] = np.asarray(inp["x_geom"], f32)
    xp = np.zeros((N, 64), f32); xp[:, :50] = np.asarray(inp["x_prior"], f32)
    xvT = np.ascontiguousarray(xv.T).astype(BF16)
    xgT = np.ascontiguousarray(xg.T).astype(BF16)
    xpT = np.ascontiguousarray(xp.T).astype(BF16)
    for c in range(NCORES):
        pc = d["percore"][c]
        pc["xvT"] = np.ascontiguousarray(xvT[:, c * NS:(c + 1) * NS])
        pc["xgT"] = np.ascontiguousarray(xgT[:, c * NS:(c + 1) * NS])
        pc["xpT"] = np.ascontiguousarray(xpT[:, c * NS:(c + 1) * NS])

    # ---- edges ----
    srcs, dsts, rels = [], [], []
    for r, key in enumerate(("edge_index_overlap", "edge_index_arch",
                             "edge_index_spatial")):
        e = np.asarray(inp[key], np.int64)
        srcs.append(e[0]); dsts.append(e[1])
        rels.append(np.full(e.shape[1], r, np.int64))
    src = np.concatenate(srcs); dst = np.concatenate(dsts)
    rel = np.concatenate(rels)
    core_of = dst // NS
    slot_of = (dst % NS) // 128
    nrel_of = (dst % 128).astype(np.int64)
    half_of = (src >= SPLIT).astype(np.int64)

    counts = np.zeros((NCORES, NSLOT, R, 2), np.int64)
    np.add.at(counts, (core_of, slot_of, rel, half_of), 1)
    K = -(-counts.max(axis=0) // 128)                     # [NSLOT,R,2]
    K = np.maximum(K, (counts.max(axis=0) > 0).astype(np.int64))

    # bucket edge ids
    keyv = ((core_of * NSLOT + slot_of) * R + rel) * 2 + half_of
    order = np.argsort(keyv, kind="stable")
    sk = keyv[order]
    bounds = np.searchsorted(sk, np.arange(NCORES * NSLOT * R * 2 + 1))

    call_cols = []
    nchunks = int(K.sum())
    for b in range(NBATCH):
        for r in range(R):
            for x in range(2):
                call_cols.append(int(K[b * BS:(b + 1) * BS, r, x].sum()) * 8)
    tot_cols = sum(call_cols)

    for c in range(NCORES):
        eidx = np.zeros((128, tot_cols), np.int16)
        mstream = np.zeros((nchunks, 128, 256), BF16)
        col0 = 0
        ci = 0
        for b in range(NBATCH):
            for r in range(R):
                for x in range(2):
                    ivs = []
                    for si in range(BS):
                        s = b * BS + si
                        kkey = ((c * NSLOT + s) * R + r) * 2 + x
                        es = order[bounds[kkey]:bounds[kkey + 1]]
                        kk = int(K[s, r, x])
                        pad = kk * 128 - len(es)
                        assert pad >= 0
                        sv = src[es] if x == 0 else src[es] - SPLIT
                        ivs.append(np.concatenate([sv, np.zeros(pad, np.int64)]))
                        nr = nrel_of[es]
                        for j in range(kk):
                            lo = j * 128
                            sub = nr[lo:lo + 128]
                            M = np.zeros((128, 128), np.float32)
                            M[np.arange(len(sub)), sub] = 1.0
                            mstream[ci + j, :, :128] = M.astype(BF16)
                            mstream[ci + j, :, 128:] = M.T.astype(BF16)
                        ci += kk
                    if ivs:
                        iv = np.concatenate(ivs)
                        ncols = len(iv) // 16
                        if ncols:
                            eidx[:, col0:col0 + ncols] = _wrap_idx(iv)
                        col0 += ncols
        assert ci == nchunks and col0 == tot_cols, (ci, nchunks, col0, tot_cols)
        d["percore"][c]["eidx"] = eidx
        d["percore"][c]["mstream"] = mstream

    # ---- LN graph ----
    batch = np.asarray(inp["batch"], np.int64)
    bc = np.bincount(batch, minlength=G)
    rcnt = (1.0 / (np.maximum(bc, 1) * F)).astype(f32)
    sh["rcnt"] = np.ascontiguousarray(rcnt.reshape(8, 128).T)
    for c in range(NCORES):
        gl = batch[c * NS:(c + 1) * NS]
        d["percore"][c]["gidx"] = _wrap_idx(gl)
        bg = np.zeros((NSLOT, 128, G), np.float32)
        bg[np.arange(NS) // 128, np.arange(NS) % 128, gl] = 1.0
        d["percore"][c]["bg"] = bg.astype(BF16)
    sh["sidx"] = _wrap_idx(np.arange(NS))
    d["K"] = K
    d["call_cols"] = call_cols
    d["nchunks"] = nchunks
    d["tot_cols"] = tot_cols
    return d


def build_kernel(pp):
    nc = bacc.Bacc("TRN2", target_bir_lowering=False, debug=False,
                   num_devices=NCORES)
    P = {}

    def param(name, shape, dt):
        P[name] = nc.dram_tensor(name, list(shape), dt, kind="ExternalInput").ap()

    param("xvT", (1024, NS), BF); param("xgT", (8, NS), BF); param("xpT", (64, NS), BF)
    param("visw", (128, 8 * 128), BF); param("gw", (8, 128), BF); param("pw", (64, 128), BF)
    for nm in ("encb", "enclw", "enclb"):
        param(nm, (128, F), F32)
    for l in range(2):
        param(f"waug{l}", (128, 3 * R * 392), BF)
        param(f"wq{l}", (128, 3 * R * 8), BF)
        for nm in (f"rb{l}", f"nw{l}", f"nb{l}"):
            param(nm, (128, F), F32)
    param("cw1", (128, 3 * 128), BF); param("cb1", (128, 1), F32)
    param("cw2", (128, 49), BF); param("cb2", (128, 49), F32)
    param("eidx", (128, pp["tot_cols"]), I16)
    param("mstream", (pp["nchunks"], 128, 256), BF)
    param("gidx", (128, NS // 16), I16)
    param("sidx", (128, NS // 16), I16)
    param("rcnt", (128, 8), F32)
    param("bg", (NSLOT, 128, G), BF)
    out_p = nc.dram_tensor("out", [NS, 49], F32, kind="ExternalOutput").ap()
    dbg_p = nc.dram_tensor("dbg", [NS, F], F32, kind="ExternalOutput").ap()
    import os
    STAGE = os.environ.get("KSTAGE", "full")
    KSUB = os.environ.get("KSUB", "all")

    K = pp["K"]; call_cols = pp["call_cols"]
    rg_all = [list(range(NCORES))]

    with tile.TileContext(nc) as tc:
        with (
            tc.tile_pool(name="const", bufs=1) as cpool,
            tc.tile_pool(name="slab", bufs=1) as slab,
            tc.tile_pool(name="work", bufs=3) as work,
            tc.tile_pool(name="gep", bufs=3) as gep,
            tc.tile_pool(name="htp", bufs=2) as htp,
            tc.tile_pool(name="sgp", bufs=1) as sgp,
            tc.tile_pool(name="mp", bufs=4) as mpool,
            tc.tile_pool(name="ps", bufs=1, space="PSUM") as pspool,
            tc.tile_pool(name="pst", bufs=2, space="PSUM") as pstmp,
            tc.tile_pool(name="dram", bufs=1, space="DRAM") as dpool,
        ):
            # ---- resident consts (2D tiles; reshape with views at use) ----
            cons = {}
            for nm, cols, dt, prows in (
                ("visw", 8 * 128, BF, 128), ("gw", 128, BF, 8), ("pw", 128, BF, 64),
                ("encb", F, F32, 128), ("enclw", F, F32, 128), ("enclb", F, F32, 128),
                ("waug0", 3 * R * 392, BF, 128), ("wq0", 3 * R * 8, BF, 128),
                ("waug1", 3 * R * 392, BF, 128), ("wq1", 3 * R * 8, BF, 128),
                ("rb0", F, F32, 128), ("nw0", F, F32, 128), ("nb0", F, F32, 128),
                ("rb1", F, F32, 128), ("nw1", F, F32, 128), ("nb1", F, F32, 128),
                ("cw1", 3 * 128, BF, 128), ("cb1", 1, F32, 128),
                ("cw2", 49, BF, 128), ("cb2", 49, F32, 128),
                ("eidx", pp["tot_cols"], I16, 128),
                ("gidx", NS // 16, I16, 128), ("sidx", NS // 16, I16, 128),
                ("rcnt", 8, F32, 128),
            ):
                t = cpool.tile([prows if prows > 1 else 1, cols], dt, tag=nm)
                nc.sync.dma_start(out=t[:prows, :], in_=P[nm][:])
                cons[nm] = t
            waugv = [cons[f"waug{l}"].rearrange("p (k r w) -> p k r w", k=3, r=R)
                     for l in range(2)]
            wqv = [cons[f"wq{l}"].rearrange("p (k r h) -> p k r h", k=3, r=R)
                   for l in range(2)]
            viswv = cons["visw"].rearrange("p (k f) -> p k f", k=8)
            cw1v = cons["cw1"].rearrange("p (k f) -> p k f", k=3)

            h_slab = slab.tile([128, NSLOT * F], BF, tag="h")
            hs = h_slab.rearrange("p (s f) -> p s f", s=NSLOT)
            gslab = slab.tile([128, 16], F32, tag="gs")

            h_local = dpool.tile([NS, F], BF, tag="hl")
            h_all = dpool.tile([N, F], BF, tag="ha")
            enc_b1 = dpool.tile([1, 8], F32, tag="eb1")
            enc_b2 = dpool.tile([1, 8], F32, tag="eb2")
            g_b1 = dpool.tile([128, 16], F32, tag="gb1")
            g_b2 = dpool.tile([128, 16], F32, tag="gb2")
            stats_t = dpool.tile([G, 64], F32, tag="st")

            ones = cpool.tile([128, 1], F32, tag="ones")
            nc.vector.memset(ones[:], 1.0)

            # ================= encoders =================
            sum6 = slab.tile([128, 6], F32, tag="s6")
            nc.vector.memset(sum6[:], 0.0)
            xvTv = P["xvT"].rearrange("(k p) n -> p k n", p=128)
            for s in range(NSLOT):
                xvt = work.tile([128, 8 * 128], BF, tag="xv")
                nc.sync.dma_start(out=xvt.rearrange("p (k n) -> p k n", k=8)[:],
                                  in_=xvTv[:, :, bass.ts(s, 128)])
                xgt = work.tile([128, 128], BF, tag="xg")
                nc.sync.dma_start(out=xgt[:8, :], in_=P["xgT"][:, bass.ts(s, 128)])
                xpt = work.tile([128, 128], BF, tag="xp")
                nc.sync.dma_start(out=xpt[:64, :], in_=P["xpT"][:, bass.ts(s, 128)])
                ps = pstmp.tile([128, 400], F32, tag="pt")
                xvtv = xvt.rearrange("p (k n) -> p k n", k=8)
                for kk in range(8):
                    nc.tensor.matmul(out=ps[:, 0:128], lhsT=xvtv[:, kk, :],
                                     rhs=viswv[:, kk, :],
                                     start=(kk == 0), stop=(kk == 7))
                nc.tensor.matmul(out=ps[:, 128:256], lhsT=xgt[:8, :],
                                 rhs=cons["gw"][:8, :], start=True, stop=True)
                nc.tensor.matmul(out=ps[:, 256:384], lhsT=xpt[:64, :],
                                 rhs=cons["pw"][:64, :], start=True, stop=True)
                hb = work.tile([128, F], F32, tag="hb")
                nc.vector.tensor_tensor(out=hb[:], in0=ps[:, 0:384],
                                        in1=cons["encb"][:],
                                        op=ALU.add)
                nc.scalar.activation(out=hb[:], in_=hb[:], func=AF.Relu)
                nc.vector.tensor_copy(out=hs[:, s, :], in_=hb[:])
                sq = work.tile([128, F], F32, tag="sq")
                nc.vector.tensor_tensor(out=sq[:], in0=hb[:], in1=hb[:], op=ALU.mult)
                r1 = work.tile([128, 3], F32, tag="r1")
                r2 = work.tile([128, 3], F32, tag="r2")
                nc.vector.tensor_reduce(out=r1[:],
                                        in_=hb.rearrange("p (b f) -> p b f", b=3)[:],
                                        axis=mybir.AxisListType.X, op=ALU.add)
                nc.vector.tensor_reduce(out=r2[:],
                                        in_=sq.rearrange("p (b f) -> p b f", b=3)[:],
                                        axis=mybir.AxisListType.X, op=ALU.add)
                nc.vector.tensor_tensor(out=sum6[:, 0:3], in0=sum6[:, 0:3],
                                        in1=r1[:], op=ALU.add)
                nc.vector.tensor_tensor(out=sum6[:, 3:6], in0=sum6[:, 3:6],
                                        in1=r2[:], op=ALU.add)
            ps6 = pstmp.tile([6, 1], F32, tag="pt")
            nc.tensor.matmul(out=ps6[:], lhsT=sum6[:], rhs=ones[:],
                             start=True, stop=True)
            s6s = work.tile([6, 1], F32, tag="s6s")
            nc.vector.tensor_copy(out=s6s[:], in_=ps6[:])
            nc.gpsimd.dma_start(out=enc_b1[0, 0:6], in_=s6s[:6, 0])
            nc.gpsimd.collective_compute("AllReduce", ALU.add,
                                         replica_groups=rg_all,
                                         ins=[enc_b1.opt()], outs=[enc_b2.opt()])
            es1 = work.tile([1, 8], F32, tag="es")
            nc.sync.dma_start(out=es1[:1, :], in_=enc_b2[:])
            ones1 = cpool.tile([128, 128], F32, tag="ones1")
            nc.vector.memset(ones1[:1, :], 1.0)
            psb = pstmp.tile([128, 400], F32, tag="pt")
            nc.tensor.matmul(out=psb[:, 0:8], lhsT=ones1[:1, :], rhs=es1[:1, :],
                             start=True, stop=True)
            es = work.tile([128, 8], F32, tag="esb")
            nc.vector.tensor_copy(out=es[:], in_=psb[:, 0:8])
            cntE = float(N * 128)
            m3 = work.tile([128, 8], F32, tag="m3")
            nc.vector.tensor_scalar_mul(m3[:, 0:3], es[:, 0:3], 1.0 / cntE)
            v3 = work.tile([128, 8], F32, tag="v3")
            nc.vector.tensor_scalar_mul(v3[:, 0:3], es[:, 3:6], 1.0 / cntE)
            q3 = work.tile([128, 8], F32, tag="q3")
            nc.vector.tensor_tensor(out=q3[:, 0:3], in0=m3[:, 0:3],
                                    in1=m3[:, 0:3], op=ALU.mult)
            nc.vector.tensor_tensor(out=v3[:, 0:3], in0=v3[:, 0:3],
                                    in1=q3[:, 0:3], op=ALU.subtract)
            nc.scalar.activation(out=v3[:, 0:3], in_=v3[:, 0:3], func=AF.Sqrt)
            nc.vector.tensor_scalar_add(v3[:, 0:3], v3[:, 0:3], EPS)
            nc.vector.reciprocal(out=v3[:, 0:3], in_=v3[:, 0:3])
            c1 = work.tile([128, F], F32, tag="c1")
            c0 = work.tile([128, F], F32, tag="c0")
            nc.vector.tensor_tensor(
                out=c1.rearrange("o (b f) -> o b f", b=3)[:],
                in0=cons["enclw"].rearrange("o (b f) -> o b f", b=3)[:],
                in1=v3[:, 0:3].to_broadcast([128, 3, 128]), op=ALU.mult)
            nc.vector.tensor_tensor(
                out=c0.rearrange("o (b f) -> o b f", b=3)[:],
                in0=c1.rearrange("o (b f) -> o b f", b=3)[:],
                in1=m3[:, 0:3].to_broadcast([128, 3, 128]), op=ALU.mult)
            nc.vector.tensor_tensor(out=c0[:], in0=cons["enclb"][:],
                                    in1=c0[:], op=ALU.subtract)
            for s in range(NSLOT):
                t = work.tile([128, F], F32, tag="hb")
                nc.vector.tensor_tensor(out=t[:], in0=hs[:, s, :],
                                        in1=c1[:], op=ALU.mult)
                nc.vector.tensor_tensor(out=t[:], in0=t[:],
                                        in1=c0[:], op=ALU.add)
                nc.vector.tensor_copy(out=hs[:, s, :], in_=t[:])
                nc.sync.dma_start(out=h_local[bass.ts(s, 128), :], in_=hs[:, s, :])

            if STAGE == "enc":
                for s in range(NSLOT):
                    t = work.tile([128, F], F32, tag="hb")
                    nc.vector.tensor_copy(out=t[:], in_=hs[:, s, :])
                    nc.sync.dma_start(out=dbg_p[bass.ts(s, 128), :], in_=t[:])
            # ================= RGAT layers =================
            NLAYERS = {"enc": 0, "l1": 1}.get(STAGE, 2)
            for l in range(NLAYERS):
                nc.gpsimd.collective_compute("AllGather", ALU.bypass,
                                             replica_groups=rg_all,
                                             ins=[h_local.opt()], outs=[h_all.opt()])
                nc.vector.memset(gslab[:], 0.0)
                ci = 0
                col0 = 0
                cci = 0
                for b in range(NBATCH):
                    hts = htp.tile([128, 3 * BS * 128], BF, tag="hts")
                    htsv = hts.rearrange("p (k e) -> p k e", k=3)
                    nc.gpsimd.dma_gather(
                        out_ap=htsv[:], in_ap=h_local[:],
                        idxs_ap=cons["sidx"][:, b * BS * 8:(b + 1) * BS * 8],
                        num_idxs=BS * 128, num_idxs_reg=BS * 128,
                        elem_size=F, transpose=True)
                    qis = work.tile([128, BS * R * 8], BF, tag="qis")
                    qisv = qis.rearrange("p (s r h) -> p s r h", s=BS, r=R)
                    for si in range(BS):
                        pq = pstmp.tile([128, 400], F32, tag="pt")
                        for r in range(R):
                            for kk in range(3):
                                nc.tensor.matmul(
                                    out=pq[:, r * 8:(r + 1) * 8],
                                    lhsT=htsv[:, kk, bass.ts(si, 128)],
                                    rhs=wqv[l][:, kk, r, :],
                                    start=(kk == 0), stop=(kk == 2))
                        nc.vector.tensor_copy(
                            out=qisv[:, si, :, :],
                            in_=pq[:, 0:R * 8].rearrange("p (r h) -> p r h", r=R)[:])
                    sink = work.tile([128, 8], F32, tag="sink")
                    ges = {}   # (r, x) -> list of (view, n_chunks)
                    GMAX = 6   # chunks per gather call (<=768 idx: HW cap ~900)
                    for r in range(R):
                        for x in range(2):
                            S16 = call_cols[cci]; cci += 1
                            S = S16 * 16
                            if S == 0 or KSUB == "qi":
                                col0 += S16
                                continue
                            nch = S // 128
                            subs = []
                            for g0 in range(0, nch, GMAX):
                                gn = min(GMAX, nch - g0)
                                Ssub = gn * 128
                                ge = gep.tile([128, 3 * Ssub], BF, tag="ge",
                                              name=f"ge{r}_{x}_{g0}")
                                src_view = (h_all[0:SPLIT + 1, :] if x == 0
                                            else h_all[SPLIT:N, :])
                                nc.gpsimd.dma_gather(
                                    out_ap=ge.rearrange("p (k e) -> p k e", k=3)[:],
                                    in_ap=src_view,
                                    idxs_ap=cons["eidx"][:, col0 + g0 * 8:
                                                         col0 + g0 * 8 + Ssub // 16],
                                    num_idxs=Ssub, num_idxs_reg=Ssub,
                                    elem_size=F, transpose=True)
                                subs.append(ge.rearrange("p (k e) -> p k e", k=3))
                                if KSUB == "gath":
                                    nc.vector.tensor_reduce(
                                        out=sink[:, 0:1], in_=ge[:, 0:128],
                                        axis=mybir.AxisListType.X, op=ALU.max)
                                    nc.sync.dma_start(
                                        out=dbg_p[bass.ts(b, 128), 0:1],
                                        in_=sink[:, 0:1])
                            ges[(r, x)] = subs
                            col0 += S16
                    upb = []
                    for si in range(BS):
                        ut = pspool.tile([128, 400], F32, tag=f"u{si}", name=f"u{b}_{si}")
                        upb.append(ut)
                    started = [False] * BS
                    if KSUB in ("qi", "gath"):
                        # consume qis so it isn't DCE'd
                        nc.gpsimd.dma_start(out=dbg_p[bass.ts(b, 128), 4:4 + BS * R * 8],
                                          in_=qis[:])
                        continue
                    # last (r, x) group with chunks, per slot (to set stop=)
                    last_rx = {}
                    for si in range(BS):
                        for r in range(R):
                            for x in range(2):
                                if int(K[b * BS + si, r, x]) > 0:
                                    last_rx[si] = (r, x)
                    for r in range(R):
                        for x in range(2):
                            subs = ges.get((r, x))
                            cl = 0
                            for si in range(BS):
                                s = b * BS + si
                                for j in range(int(K[s, r, x])):
                                    gev = subs[cl // 6]
                                    off = (cl % 6) * 128
                                    cl += 1
                                    mp = mpool.tile([128, 256], BF, tag="mp")
                                    nc.sync.dma_start(out=mp[:],
                                                      in_=P["mstream"][ci, :, :])
                                    pt = pstmp.tile([128, 400], F32, tag="pt")
                                    for kk in range(3):
                                        nc.tensor.matmul(
                                            out=pt[:, 0:392],
                                            lhsT=gev[:, kk, off:off + 128],
                                            rhs=waugv[l][:, kk, r, :],
                                            start=(kk == 0), stop=(kk == 2))
                                    nc.tensor.matmul(
                                        out=pt[:, 392:400], lhsT=mp[:, 128:256],
                                        rhs=qisv[:, si, r, :], start=True, stop=True)
                                    qe = work.tile([128, 8], F32, tag="qe")
                                    nc.vector.tensor_copy(out=qe[:], in_=pt[:, 392:400])
                                    at = work.tile([128, 8], F32, tag="at")
                                    nc.vector.tensor_tensor(
                                        out=at[:], in0=pt[:, 384:392],
                                        in1=qe[:], op=ALU.add)
                                    at2 = work.tile([128, 8], F32, tag="at2")
                                    nc.vector.tensor_scalar_mul(at2[:], at[:], NEG)
                                    nc.vector.tensor_tensor(out=at[:], in0=at[:],
                                                            in1=at2[:], op=ALU.max)
                                    nc.scalar.activation(out=at[:], in_=at[:],
                                                         func=AF.Exp)
                                    me = work.tile([128, 392], BF, tag="me")
                                    nc.vector.tensor_tensor(
                                        out=me[:, 0:384].rearrange(
                                            "p (h c) -> p h c", h=H)[:],
                                        in0=pt[:, 0:384].rearrange(
                                            "p (h c) -> p h c", h=H)[:],
                                        in1=at[:].to_broadcast([128, H, C]),
                                        op=ALU.mult)
                                    nc.vector.tensor_copy(out=me[:, 384:392],
                                                          in_=at[:])
                                    is_last = (last_rx.get(si) == (r, x)
                                               and j == int(K[s, r, x]) - 1)
                                    nc.tensor.matmul(
                                        out=upb[si][:, 0:392], lhsT=mp[:, 0:128],
                                        rhs=me[:], start=not started[si],
                                        stop=is_last)
                                    started[si] = True
                                    ci += 1
                    if KSUB == "chunk":
                        for si in range(BS):
                            uo = work.tile([128, 400], F32, tag="uo")
                            nc.vector.tensor_copy(out=uo[:], in_=upb[si][:])
                            nc.sync.dma_start(out=dbg_p[bass.ts(b, 128), 0:384],
                                              in_=uo[:, 0:384])
                        continue
                    for si in range(BS):
                        s = b * BS + si
                        up = upb[si]
                        if not started[si]:
                            nc.vector.memset(up[:], 0.0)
                        sr = work.tile([128, 8], F32, tag="sr")
                        nc.vector.tensor_scalar_add(sr[:], up[:, 384:392], 1e-16)
                        nc.vector.reciprocal(out=sr[:], in_=sr[:])
                        z = work.tile([128, F], F32, tag="z")
                        nc.vector.tensor_tensor(
                            out=z.rearrange("p (h c) -> p h c", h=H)[:],
                            in0=up[:, 0:384].rearrange("p (h c) -> p h c", h=H)[:],
                            in1=sr[:].to_broadcast([128, H, C]), op=ALU.mult)
                        nc.vector.tensor_tensor(
                            out=z[:], in0=z[:],
                            in1=cons[f"rb{l}"][:],
                            op=ALU.add)
                        e1 = work.tile([128, F], F32, tag="e1")
                        nc.vector.tensor_scalar_min(e1[:], z[:], 0.0)
                        nc.scalar.activation(out=e1[:], in_=e1[:], func=AF.Exp)
                        nc.vector.tensor_scalar_add(e1[:], e1[:], -1.0)
                        nc.vector.tensor_tensor(out=z[:], in0=z[:], in1=e1[:],
                                                op=ALU.max)
                        nc.vector.tensor_tensor(out=z[:], in0=z[:],
                                                in1=hs[:, s, :], op=ALU.add)
                        nc.vector.tensor_copy(out=hs[:, s, :], in_=z[:])
                        rs = work.tile([128, 2], F32, tag="rs")
                        nc.vector.tensor_reduce(out=rs[:, 0:1], in_=z[:],
                                                axis=mybir.AxisListType.X,
                                                op=ALU.add)
                        sq2 = work.tile([128, F], F32, tag="sq")
                        nc.vector.tensor_tensor(out=sq2[:], in0=z[:], in1=z[:],
                                                op=ALU.mult)
                        nc.vector.tensor_reduce(out=rs[:, 1:2], in_=sq2[:],
                                                axis=mybir.AxisListType.X,
                                                op=ALU.add)
                        rsb = work.tile([128, 2], BF, tag="rsb")
                        nc.vector.tensor_copy(out=rsb[:], in_=rs[:])
                        bgt = work.tile([128, 8 * 128], BF, tag="bgt")
                        bgtv = bgt.rearrange("p (j g) -> p j g", j=8)
                        nc.sync.dma_start(
                            out=bgtv[:],
                            in_=P["bg"][s].rearrange("p (j g) -> p j g", j=8))
                        for jb in range(8):
                            nc.tensor.matmul(out=up[:, 16 * jb:16 * jb + 2],
                                             lhsT=bgtv[:, jb, :], rhs=rsb[:],
                                             start=True, stop=True)
                        gj = work.tile([128, 16], F32, tag="gj")
                        nc.vector.tensor_copy(
                            out=gj.rearrange("p (j e) -> p j e", j=8)[:],
                            in_=up[:, 0:128].rearrange(
                                "p (j e) -> p j e", j=8)[:, :, 0:2])
                        nc.vector.tensor_tensor(out=gslab[:], in0=gslab[:],
                                                in1=gj[:], op=ALU.add)
                if KSUB != "all":
                    continue
                nc.gpsimd.dma_start(out=g_b1[:], in_=gslab[:])
                nc.gpsimd.collective_compute("AllReduce", ALU.add,
                                             replica_groups=rg_all,
                                             ins=[g_b1.opt()], outs=[g_b2.opt()])
                gr = work.tile([128, 16], F32, tag="gr")
                nc.sync.dma_start(out=gr[:], in_=g_b2[:])
                grv = gr.rearrange("p (j e) -> p j e", j=8)
                mean = work.tile([128, 8], F32, tag="mean")
                nc.vector.tensor_tensor(out=mean[:], in0=grv[:, :, 0],
                                        in1=cons["rcnt"][:], op=ALU.mult)
                ex2 = work.tile([128, 8], F32, tag="ex2")
                nc.vector.tensor_tensor(out=ex2[:], in0=grv[:, :, 1],
                                        in1=cons["rcnt"][:], op=ALU.mult)
                msq = work.tile([128, 8], F32, tag="msq")
                nc.vector.tensor_tensor(out=msq[:], in0=mean[:], in1=mean[:],
                                        op=ALU.mult)
                nc.vector.tensor_tensor(out=ex2[:], in0=ex2[:], in1=msq[:],
                                        op=ALU.subtract)
                nc.vector.tensor_scalar_add(ex2[:], ex2[:], EPS)
                nc.scalar.activation(out=ex2[:], in_=ex2[:], func=AF.Sqrt)
                nc.vector.reciprocal(out=ex2[:], in_=ex2[:])
                stw = work.tile([128, 16], F32, tag="stw")
                stwv = stw.rearrange("p (j e) -> p j e", j=8)
                nc.vector.tensor_copy(out=stwv[:, :, 0], in_=mean[:])
                nc.vector.tensor_copy(out=stwv[:, :, 1], in_=ex2[:])
                nc.sync.dma_start(
                    out=stats_t.rearrange("(j p) e -> p j e", p=128)[:, :, 0:2],
                    in_=stwv[:])
                stg = sgp.tile([128, NSLOT * 64], F32, tag="stg")
                stgv = stg.rearrange("p (s e) -> p s e", s=NSLOT)
                for gb in range(NBATCH):
                    nc.gpsimd.dma_gather(
                        out_ap=stgv[:, gb * BS:(gb + 1) * BS, :],
                        in_ap=stats_t[:],
                        idxs_ap=cons["gidx"][:, gb * BS * 8:(gb + 1) * BS * 8],
                        num_idxs=BS * 128, num_idxs_reg=BS * 128,
                        elem_size=64, transpose=False)
                for s in range(NSLOT):
                    t = work.tile([128, F], F32, tag="z")
                    nc.vector.tensor_scalar(
                        out=t[:], in0=hs[:, s, :],
                        scalar1=stgv[:, s, 0:1], scalar2=stgv[:, s, 1:2],
                        op0=ALU.subtract, op1=ALU.mult)
                    nc.vector.tensor_tensor(
                        out=t[:], in0=t[:],
                        in1=cons[f"nw{l}"][:],
                        op=ALU.mult)
                    nc.vector.tensor_tensor(
                        out=t[:], in0=t[:],
                        in1=cons[f"nb{l}"][:],
                        op=ALU.add)
                    nc.vector.tensor_copy(out=hs[:, s, :], in_=t[:])
                    nc.sync.dma_start(out=h_local[bass.ts(s, 128), :],
                                      in_=hs[:, s, :])

            if STAGE == "l1":
                for s in range(NSLOT):
                    t = work.tile([128, F], F32, tag="hb")
                    nc.vector.tensor_copy(out=t[:], in_=hs[:, s, :])
                    nc.sync.dma_start(out=dbg_p[bass.ts(s, 128), :], in_=t[:])
            # ================= head =================
            for b in range(NBATCH if STAGE == "full" else 0):
                hts = htp.tile([128, 3 * BS * 128], BF, tag="hts")
                htsv = hts.rearrange("p (k e) -> p k e", k=3)
                nc.gpsimd.dma_gather(
                    out_ap=htsv[:], in_ap=h_local[:],
                    idxs_ap=cons["sidx"][:, b * BS * 8:(b + 1) * BS * 8],
                    num_idxs=BS * 128, num_idxs_reg=BS * 128,
                    elem_size=F, transpose=True)
                for si in range(BS):
                    s = b * BS + si
                    pz = pstmp.tile([128, 400], F32, tag="pt")
                    for kk in range(3):
                        nc.tensor.matmul(out=pz[:, 0:128], lhsT=cw1v[:, kk, :],
                                         rhs=htsv[:, kk, bass.ts(si, 128)],
                                         start=(kk == 0), stop=(kk == 2))
                    z1 = work.tile([128, 128], BF, tag="z1")
                    nc.scalar.activation(out=z1[:], in_=pz[:, 0:128],
                                         func=AF.Relu, bias=cons["cb1"][:])
                    po = pstmp.tile([128, 400], F32, tag="pt")
                    nc.tensor.matmul(out=po[:, 0:49], lhsT=z1[:],
                                     rhs=cons["cw2"][:], start=True, stop=True)
                    ot = work.tile([128, 49], F32, tag="ot")
                    nc.vector.tensor_tensor(
                        out=ot[:], in0=po[:, 0:49],
                        in1=cons["cb2"][:],
                        op=ALU.add)
                    nc.sync.dma_start(out=out_p[bass.ts(s, 128), :], in_=ot[:])

    nc.compile()
    return nc


def kernel(**inputs):
    pp = preprocess(inputs)
    nc = build_kernel(pp)
    in_maps = []
    for c in range(NCORES):
        m = dict(pp["shared"])
        m.update(pp["percore"][c])
        in_maps.append(m)
    res = run_bass_kernel_spmd(nc, in_maps, core_ids=list(range(NCORES)))
    out = np.concatenate([res.results[c]["out"] for c in range(NCORES)], axis=0)
    return out.astype(np.float32)


if __name__ == "__main__":
    import time
    import jax
    import reference
    t0 = time.perf_counter()
    with jax.default_device(jax.devices("cpu")[0]):
        inputs = {k: np.asarray(v) for k, v in reference.setup_inputs().items()}
        exp = np.asarray(reference.reference(**inputs))
    print(f"reference done in {time.perf_counter()-t0:.1f}s")
    t0 = time.perf_counter()
    got = kernel(**inputs)
    print(f"kernel done in {time.perf_counter()-t0:.1f}s")
    rel = np.linalg.norm(got - exp) / (np.linalg.norm(exp) + 1e-30)
    mx = np.abs(got - exp).max()
    print(f"Relative error: {rel:.4e}   max-abs: {mx:.3e}  exp-scale: {np.abs(exp).max():.3f}")


# revision 20
# speedup vs baseline: 15.7085x; 15.7085x over previous
"""AnatomyGAT (2-layer RGAT over 1024 graphs) on 8 TRN2 NeuronCores, Bass/Tile.

Sharding: node-parallel. Core c owns nodes [c*6144,(c+1)*6144); edges live on
the dst-owner core, grouped per (dst slot of 128 nodes, relation, src-half)
into 128-edge chunks with a chunk structure that is the max over cores (SPMD
static program; pads use dummy src index 0 and zero rows in the one-hot M).

Per chunk: transpose dma_gather of h[src] (bf16) -> TensorE per-edge
transform [oj|kj] = h_src @ [W_r|W_r k] -> w = exp(lrelu(qi[dst]+kj)) with
qi[dst] expanded by M^T matmul -> U += M^T @ [w*oj | w] in PSUM per slot.
Segment softmax denominator folded in at node level: U/(S+1e-16).
h is AllGathered (bf16) once per layer; per-graph LN stats via one-hot
matmuls + AllReduce + stats-table gather by batch id.
"""

import numpy as np
import ml_dtypes

import concourse.bass as bass
import concourse.bacc as bacc
import concourse.mybir as mybir
import concourse.tile as tile
from concourse.bass_utils import run_bass_kernel_spmd

BF16 = ml_dtypes.bfloat16
F32 = mybir.dt.float32
BF = mybir.dt.bfloat16
I16 = mybir.dt.int16

N, G, R, H, C, F = 49152, 1024, 3, 8, 48, 384
NCORES = 8
NS = N // NCORES          # 6144
NSLOT = NS // 128         # 48
NBATCH = 8
BS = NSLOT // NBATCH      # 6
SPLIT = 32767
NEG = 0.2
EPS = 1e-5
AF = mybir.ActivationFunctionType
ALU = mybir.AluOpType


def _wrap_idx(idx):
    idx = np.asarray(idx, np.int16)
    assert len(idx) % 16 == 0
    return np.tile(idx.reshape(-1, 16).T, (8, 1))


def preprocess(inp):
    f32 = np.float32
    d = {"shared": {}, "percore": [dict() for _ in range(NCORES)]}
    sh = d["shared"]

    # ---- weights ----
    for l, pfx in ((0, "r1"), (1, "r2")):
        W = np.asarray(inp[f"{pfx}_w"], f32)              # [R,384,384]
        q = np.asarray(inp[f"{pfx}_q"], f32)              # [384,8]
        k = np.asarray(inp[f"{pfx}_k"], f32)
        waug = np.concatenate([W, W @ k], axis=2)         # [R,384,392]
        # store [128, kchunk(3), r(3), 392]
        sh[f"waug{l}"] = np.ascontiguousarray(
            waug.reshape(R, 3, 128, 392).transpose(2, 1, 0, 3)
            .reshape(128, 3 * R * 392)).astype(BF16)
        wq = W @ q                                        # [R,384,8]
        sh[f"wq{l}"] = np.ascontiguousarray(
            wq.reshape(R, 3, 128, 8).transpose(2, 1, 0, 3)
            .reshape(128, 3 * R * 8)).astype(BF16)
        sh[f"rb{l}"] = np.repeat(np.asarray(inp[f"{pfx}_b"], f32).reshape(1, F), 128, 0)
        sh[f"nw{l}"] = np.repeat(np.asarray(inp[f"n{l+1}_w"], f32).reshape(1, F), 128, 0)
        sh[f"nb{l}"] = np.repeat(np.asarray(inp[f"n{l+1}_b"], f32).reshape(1, F), 128, 0)

    vis_w = np.asarray(inp["vis_w"], f32)                 # [1024,128]
    sh["visw"] = np.ascontiguousarray(
        vis_w.reshape(8, 128, 128).transpose(1, 0, 2).reshape(128, 8 * 128)).astype(BF16)
    gw = np.zeros((8, 128), f32); gw[:6] = np.asarray(inp["geom_w"], f32)
    sh["gw"] = gw.astype(BF16)
    pw = np.zeros((64, 128), f32); pw[:50] = np.asarray(inp["prior_w"], f32)
    sh["pw"] = pw.astype(BF16)
    sh["encb"] = np.repeat(np.concatenate([np.asarray(inp["vis_b"], f32),
                                 np.asarray(inp["geom_b"], f32),
                                 np.asarray(inp["prior_b"], f32)]).reshape(1, F), 128, 0)
    sh["enclw"] = np.repeat(np.concatenate([np.asarray(inp["vis_lw"], f32),
                                  np.asarray(inp["geom_lw"], f32),
                                  np.asarray(inp["prior_lw"], f32)]).reshape(1, F), 128, 0)
    sh["enclb"] = np.repeat(np.concatenate([np.asarray(inp["vis_lb"], f32),
                                  np.asarray(inp["geom_lb"], f32),
                                  np.asarray(inp["prior_lb"], f32)]).reshape(1, F), 128, 0)
    cw1 = np.asarray(inp["c_w1"], f32)                    # [384,128]
    sh["cw1"] = np.ascontiguousarray(
        cw1.reshape(3, 128, 128).transpose(1, 0, 2).reshape(128, 3 * 128)).astype(BF16)
    sh["cb1"] = np.asarray(inp["c_b1"], f32).reshape(128, 1)
    sh["cw2"] = np.asarray(inp["c_w2"], f32).astype(BF16)
    sh["cb2"] = np.repeat(np.asarray(inp["c_b2"], f32).reshape(1, 49), 128, 0)

    # ---- feature shards (transposed, bf16) ----
    xv = np.asarray(inp["x_visual"], f32)
    xg = np.zeros((N, 8), f32); xg[:, :6] = np.asarray(inp["x_geom"], f32)
    xp = np.zeros((N, 64), f32); xp[:, :50] = np.asarray(inp["x_prior"], f32)
    xvT = np.ascontiguousarray(xv.T).astype(BF16)
    xgT = np.ascontiguousarray(xg.T).astype(BF16)
    xpT = np.ascontiguousarray(xp.T).astype(BF16)
    for c in range(NCORES):
        pc = d["percore"][c]
        pc["xvT"] = np.ascontiguousarray(xvT[:, c * NS:(c + 1) * NS])
        pc["xgT"] = np.ascontiguousarray(xgT[:, c * NS:(c + 1) * NS])
        pc["xpT"] = np.ascontiguousarray(xpT[:, c * NS:(c + 1) * NS])

    # ---- edges ----
    srcs, dsts, rels = [], [], []
    for r, key in enumerate(("edge_index_overlap", "edge_index_arch",
                             "edge_index_spatial")):
        e = np.asarray(inp[key], np.int64)
        srcs.append(e[0]); dsts.append(e[1])
        rels.append(np.full(e.shape[1], r, np.int64))
    src = np.concatenate(srcs); dst = np.concatenate(dsts)
    rel = np.concatenate(rels)
    core_of = dst // NS
    slot_of = (dst % NS) // 128
    nrel_of = (dst % 128).astype(np.int64)
    half_of = (src >= SPLIT).astype(np.int64)

    counts = np.zeros((NCORES, NSLOT, R, 2), np.int64)
    np.add.at(counts, (core_of, slot_of, rel, half_of), 1)
    K = -(-counts.max(axis=0) // 128)                     # [NSLOT,R,2]
    K = np.maximum(K, (counts.max(axis=0) > 0).astype(np.int64))

    # bucket edge ids
    keyv = ((core_of * NSLOT + slot_of) * R + rel) * 2 + half_of
    order = np.argsort(keyv, kind="stable")
    sk = keyv[order]
    bounds = np.searchsorted(sk, np.arange(NCORES * NSLOT * R * 2 + 1))

    call_cols = []
    nchunks = int(K.sum())
    for b in range(NBATCH):
        for r in range(R):
            for x in range(2):
                call_cols.append(int(K[b * BS:(b + 1) * BS, r, x].sum()) * 8)
    tot_cols = sum(call_cols)

    for c in range(NCORES):
        eidx = np.zeros((128, tot_cols), np.int16)
        mstream = np.zeros((nchunks, 128, 256), BF16)
        col0 = 0
        ci = 0
        for b in range(NBATCH):
            for r in range(R):
                for x in range(2):
                    ivs = []
                    for si in range(BS):
                        s = b * BS + si
                        kkey = ((c * NSLOT + s) * R + r) * 2 + x
                        es = order[bounds[kkey]:bounds[kkey + 1]]
                        kk = int(K[s, r, x])
                        pad = kk * 128 - len(es)
                        assert pad >= 0
                        sv = src[es] if x == 0 else src[es] - SPLIT
                        ivs.append(np.concatenate([sv, np.zeros(pad, np.int64)]))
                        nr = nrel_of[es]
                        for j in range(kk):
                            lo = j * 128
                            sub = nr[lo:lo + 128]
                            M = np.zeros((128, 128), np.float32)
                            M[np.arange(len(sub)), sub] = 1.0
                            mstream[ci + j, :, :128] = M.astype(BF16)
                            mstream[ci + j, :, 128:] = M.T.astype(BF16)
                        ci += kk
                    if ivs:
                        iv = np.concatenate(ivs)
                        ncols = len(iv) // 16
                        if ncols:
                            eidx[:, col0:col0 + ncols] = _wrap_idx(iv)
                        col0 += ncols
        assert ci == nchunks and col0 == tot_cols, (ci, nchunks, col0, tot_cols)
        d["percore"][c]["eidx"] = eidx
        d["percore"][c]["mstream"] = mstream

    # ---- LN graph ----
    batch = np.asarray(inp["batch"], np.int64)
    bc = np.bincount(batch, minlength=G)
    rcnt = (1.0 / (np.maximum(bc, 1) * F)).astype(f32)
    sh["rcnt"] = np.ascontiguousarray(rcnt.reshape(8, 128).T)
    for c in range(NCORES):
        gl = batch[c * NS:(c + 1) * NS]
        d["percore"][c]["gidx"] = _wrap_idx(gl)
        bg = np.zeros((NSLOT, 128, G), np.float32)
        bg[np.arange(NS) // 128, np.arange(NS) % 128, gl] = 1.0
        d["percore"][c]["bg"] = bg.astype(BF16)
    sh["sidx"] = _wrap_idx(np.arange(NS))
    d["K"] = K
    d["call_cols"] = call_cols
    d["nchunks"] = nchunks
    d["tot_cols"] = tot_cols
    return d


def build_kernel(pp):
    nc = bacc.Bacc("TRN2", target_bir_lowering=False, debug=False,
                   num_devices=NCORES)
    P = {}

    def param(name, shape, dt):
        P[name] = nc.dram_tensor(name, list(shape), dt, kind="ExternalInput").ap()

    param("xvT", (1024, NS), BF); param("xgT", (8, NS), BF); param("xpT", (64, NS), BF)
    param("visw", (128, 8 * 128), BF); param("gw", (8, 128), BF); param("pw", (64, 128), BF)
    for nm in ("encb", "enclw", "enclb"):
        param(nm, (128, F), F32)
    for l in range(2):
        param(f"waug{l}", (128, 3 * R * 392), BF)
        param(f"wq{l}", (128, 3 * R * 8), BF)
        for nm in (f"rb{l}", f"nw{l}", f"nb{l}"):
            param(nm, (128, F), F32)
    param("cw1", (128, 3 * 128), BF); param("cb1", (128, 1), F32)
    param("cw2", (128, 49), BF); param("cb2", (128, 49), F32)
    param("eidx", (128, pp["tot_cols"]), I16)
    param("mstream", (pp["nchunks"], 128, 256), BF)
    param("gidx", (128, NS // 16), I16)
    param("sidx", (128, NS // 16), I16)
    param("rcnt", (128, 8), F32)
    param("bg", (NSLOT, 128, G), BF)
    out_p = nc.dram_tensor("out", [NS, 49], F32, kind="ExternalOutput").ap()
    dbg_p = nc.dram_tensor("dbg", [NS, F], F32, kind="ExternalOutput").ap()
    import os
    STAGE = os.environ.get("KSTAGE", "full")
    KSUB = os.environ.get("KSUB", "all")
    WB = int(os.environ.get("KWB", "3"))
    MB = int(os.environ.get("KMB", "4"))
    GB = int(os.environ.get("KGB", "3"))

    K = pp["K"]; call_cols = pp["call_cols"]
    rg_all = [list(range(NCORES))]

    with tile.TileContext(nc) as tc:
        with (
            tc.tile_pool(name="const", bufs=1) as cpool,
            tc.tile_pool(name="slab", bufs=1) as slab,
            tc.tile_pool(name="work", bufs=WB) as work,
            tc.tile_pool(name="gep", bufs=GB) as gep,
            tc.tile_pool(name="htp", bufs=2) as htp,
            tc.tile_pool(name="sgp", bufs=1) as sgp,
            tc.tile_pool(name="mp", bufs=MB) as mpool,
            tc.tile_pool(name="ps", bufs=1, space="PSUM") as pspool,
            tc.tile_pool(name="pst", bufs=2, space="PSUM") as pstmp,
            tc.tile_pool(name="dram", bufs=1, space="DRAM") as dpool,
        ):
            # ---- resident consts (2D tiles; reshape with views at use) ----
            cons = {}
            for nm, cols, dt, prows in (
                ("visw", 8 * 128, BF, 128), ("gw", 128, BF, 8), ("pw", 128, BF, 64),
                ("encb", F, F32, 128), ("enclw", F, F32, 128), ("enclb", F, F32, 128),
                ("waug0", 3 * R * 392, BF, 128), ("wq0", 3 * R * 8, BF, 128),
                ("waug1", 3 * R * 392, BF, 128), ("wq1", 3 * R * 8, BF, 128),
                ("rb0", F, F32, 128), ("nw0", F, F32, 128), ("nb0", F, F32, 128),
                ("rb1", F, F32, 128), ("nw1", F, F32, 128), ("nb1", F, F32, 128),
                ("cw1", 3 * 128, BF, 128), ("cb1", 1, F32, 128),
                ("cw2", 49, BF, 128), ("cb2", 49, F32, 128),
                ("eidx", pp["tot_cols"], I16, 128),
                ("gidx", NS // 16, I16, 128), ("sidx", NS // 16, I16, 128),
                ("rcnt", 8, F32, 128),
            ):
                t = cpool.tile([prows if prows > 1 else 1, cols], dt, tag=nm)
                nc.sync.dma_start(out=t[:prows, :], in_=P[nm][:])
                cons[nm] = t
            waugv = [cons[f"waug{l}"].rearrange("p (k r w) -> p k r w", k=3, r=R)
                     for l in range(2)]
            wqv = [cons[f"wq{l}"].rearrange("p (k r h) -> p k r h", k=3, r=R)
                   for l in range(2)]
            viswv = cons["visw"].rearrange("p (k f) -> p k f", k=8)
            cw1v = cons["cw1"].rearrange("p (k f) -> p k f", k=3)

            h_slab = slab.tile([128, NSLOT * F], BF, tag="h")
            hs = h_slab.rearrange("p (s f) -> p s f", s=NSLOT)
            gslab = slab.tile([128, 16], F32, tag="gs")

            h_local = dpool.tile([NS, F], BF, tag="hl")
            h_all = dpool.tile([N, F], BF, tag="ha")
            enc_b1 = dpool.tile([1, 8], F32, tag="eb1")
            enc_b2 = dpool.tile([1, 8], F32, tag="eb2")
            g_b1 = dpool.tile([128, 16], F32, tag="gb1")
            g_b2 = dpool.tile([128, 16], F32, tag="gb2")
            stats_t = dpool.tile([G, 64], F32, tag="st")

            ones = cpool.tile([128, 1], F32, tag="ones")
            nc.vector.memset(ones[:], 1.0)

            # ================= encoders =================
            sum6 = slab.tile([128, 6], F32, tag="s6")
            nc.vector.memset(sum6[:], 0.0)
            xvTv = P["xvT"].rearrange("(k p) n -> p k n", p=128)
            for s in range(NSLOT):
                xvt = work.tile([128, 8 * 128], BF, tag="xv")
                nc.sync.dma_start(out=xvt.rearrange("p (k n) -> p k n", k=8)[:],
                                  in_=xvTv[:, :, bass.ts(s, 128)])
                xgt = work.tile([128, 128], BF, tag="xg")
                nc.sync.dma_start(out=xgt[:8, :], in_=P["xgT"][:, bass.ts(s, 128)])
                xpt = work.tile([128, 128], BF, tag="xp")
                nc.sync.dma_start(out=xpt[:64, :], in_=P["xpT"][:, bass.ts(s, 128)])
                ps = pstmp.tile([128, 400], F32, tag="pt")
                xvtv = xvt.rearrange("p (k n) -> p k n", k=8)
                for kk in range(8):
                    nc.tensor.matmul(out=ps[:, 0:128], lhsT=xvtv[:, kk, :],
                                     rhs=viswv[:, kk, :],
                                     start=(kk == 0), stop=(kk == 7))
                nc.tensor.matmul(out=ps[:, 128:256], lhsT=xgt[:8, :],
                                 rhs=cons["gw"][:8, :], start=True, stop=True)
                nc.tensor.matmul(out=ps[:, 256:384], lhsT=xpt[:64, :],
                                 rhs=cons["pw"][:64, :], start=True, stop=True)
                hb = work.tile([128, F], F32, tag="hb")
                nc.vector.tensor_tensor(out=hb[:], in0=ps[:, 0:384],
                                        in1=cons["encb"][:],
                                        op=ALU.add)
                nc.scalar.activation(out=hb[:], in_=hb[:], func=AF.Relu)
                nc.vector.tensor_copy(out=hs[:, s, :], in_=hb[:])
                sq = work.tile([128, F], F32, tag="sq")
                nc.vector.tensor_tensor(out=sq[:], in0=hb[:], in1=hb[:], op=ALU.mult)
                r1 = work.tile([128, 3], F32, tag="r1")
                r2 = work.tile([128, 3], F32, tag="r2")
                nc.vector.tensor_reduce(out=r1[:],
                                        in_=hb.rearrange("p (b f) -> p b f", b=3)[:],
                                        axis=mybir.AxisListType.X, op=ALU.add)
                nc.vector.tensor_reduce(out=r2[:],
                                        in_=sq.rearrange("p (b f) -> p b f", b=3)[:],
                                        axis=mybir.AxisListType.X, op=ALU.add)
                nc.vector.tensor_tensor(out=sum6[:, 0:3], in0=sum6[:, 0:3],
                                        in1=r1[:], op=ALU.add)
                nc.vector.tensor_tensor(out=sum6[:, 3:6], in0=sum6[:, 3:6],
                                        in1=r2[:], op=ALU.add)
            ps6 = pstmp.tile([6, 1], F32, tag="pt")
            nc.tensor.matmul(out=ps6[:], lhsT=sum6[:], rhs=ones[:],
                             start=True, stop=True)
            s6s = work.tile([6, 1], F32, tag="s6s")
            nc.vector.tensor_copy(out=s6s[:], in_=ps6[:])
            nc.gpsimd.dma_start(out=enc_b1[0, 0:6], in_=s6s[:6, 0])
            nc.gpsimd.collective_compute("AllReduce", ALU.add,
                                         replica_groups=rg_all,
                                         ins=[enc_b1.opt()], outs=[enc_b2.opt()])
            es1 = work.tile([1, 8], F32, tag="es")
            nc.sync.dma_start(out=es1[:1, :], in_=enc_b2[:])
            ones1 = cpool.tile([128, 128], F32, tag="ones1")
            nc.vector.memset(ones1[:1, :], 1.0)
            psb = pstmp.tile([128, 400], F32, tag="pt")
            nc.tensor.matmul(out=psb[:, 0:8], lhsT=ones1[:1, :], rhs=es1[:1, :],
                             start=True, stop=True)
            es = work.tile([128, 8], F32, tag="esb")
            nc.vector.tensor_copy(out=es[:], in_=psb[:, 0:8])
            cntE = float(N * 128)
            m3 = work.tile([128, 8], F32, tag="m3")
            nc.vector.tensor_scalar_mul(m3[:, 0:3], es[:, 0:3], 1.0 / cntE)
            v3 = work.tile([128, 8], F32, tag="v3")
            nc.vector.tensor_scalar_mul(v3[:, 0:3], es[:, 3:6], 1.0 / cntE)
            q3 = work.tile([128, 8], F32, tag="q3")
            nc.vector.tensor_tensor(out=q3[:, 0:3], in0=m3[:, 0:3],
                                    in1=m3[:, 0:3], op=ALU.mult)
            nc.vector.tensor_tensor(out=v3[:, 0:3], in0=v3[:, 0:3],
                                    in1=q3[:, 0:3], op=ALU.subtract)
            nc.scalar.activation(out=v3[:, 0:3], in_=v3[:, 0:3], func=AF.Sqrt)
            nc.vector.tensor_scalar_add(v3[:, 0:3], v3[:, 0:3], EPS)
            nc.vector.reciprocal(out=v3[:, 0:3], in_=v3[:, 0:3])
            c1 = work.tile([128, F], F32, tag="c1")
            c0 = work.tile([128, F], F32, tag="c0")
            nc.vector.tensor_tensor(
                out=c1.rearrange("o (b f) -> o b f", b=3)[:],
                in0=cons["enclw"].rearrange("o (b f) -> o b f", b=3)[:],
                in1=v3[:, 0:3].to_broadcast([128, 3, 128]), op=ALU.mult)
            nc.vector.tensor_tensor(
                out=c0.rearrange("o (b f) -> o b f", b=3)[:],
                in0=c1.rearrange("o (b f) -> o b f", b=3)[:],
                in1=m3[:, 0:3].to_broadcast([128, 3, 128]), op=ALU.mult)
            nc.vector.tensor_tensor(out=c0[:], in0=cons["enclb"][:],
                                    in1=c0[:], op=ALU.subtract)
            for s in range(NSLOT):
                t = work.tile([128, F], F32, tag="hb")
                nc.vector.tensor_tensor(out=t[:], in0=hs[:, s, :],
                                        in1=c1[:], op=ALU.mult)
                nc.vector.tensor_tensor(out=t[:], in0=t[:],
                                        in1=c0[:], op=ALU.add)
                nc.vector.tensor_copy(out=hs[:, s, :], in_=t[:])
                nc.sync.dma_start(out=h_local[bass.ts(s, 128), :], in_=hs[:, s, :])

            if STAGE == "enc":
                for s in range(NSLOT):
                    t = work.tile([128, F], F32, tag="hb")
                    nc.vector.tensor_copy(out=t[:], in_=hs[:, s, :])
                    nc.sync.dma_start(out=dbg_p[bass.ts(s, 128), :], in_=t[:])
            # ================= RGAT layers =================
            NLAYERS = {"enc": 0, "l1": 1}.get(STAGE, 2)
            for l in range(NLAYERS):
                nc.gpsimd.collective_compute("AllGather", ALU.bypass,
                                             replica_groups=rg_all,
                                             ins=[h_local.opt()], outs=[h_all.opt()])
                nc.vector.memset(gslab[:], 0.0)
                ci = 0
                col0 = 0
                cci = 0
                for b in range(NBATCH):
                    hts = htp.tile([128, 3 * BS * 128], BF, tag="hts")
                    htsv = hts.rearrange("p (k e) -> p k e", k=3)
                    nc.gpsimd.dma_gather(
                        out_ap=htsv[:], in_ap=h_local[:],
                        idxs_ap=cons["sidx"][:, b * BS * 8:(b + 1) * BS * 8],
                        num_idxs=BS * 128, num_idxs_reg=BS * 128,
                        elem_size=F, transpose=True)
                    qis = work.tile([128, BS * R * 8], BF, tag="qis")
                    qisv = qis.rearrange("p (s r h) -> p s r h", s=BS, r=R)
                    for si in range(BS):
                        pq = pstmp.tile([128, 400], F32, tag="pt")
                        for kk in range(3):
                            nc.tensor.matmul(
                                out=pq[:, 0:R * 8],
                                lhsT=htsv[:, kk, bass.ts(si, 128)],
                                rhs=cons[f"wq{l}"][:, kk * R * 8:(kk + 1) * R * 8],
                                start=(kk == 0), stop=(kk == 2))
                        nc.vector.tensor_copy(
                            out=qisv[:, si, :, :],
                            in_=pq[:, 0:R * 8].rearrange("p (r h) -> p r h", r=R)[:])
                    sink = work.tile([128, 8], F32, tag="sink")
                    ges = {}   # (r, x) -> list of (view, n_chunks)
                    GMAX = int(__import__("os").environ.get("KGMAX", "6"))  # <=7: HW ring cap ~900 idx/call
                    for r in range(R):
                        for x in range(2):
                            S16 = call_cols[cci]; cci += 1
                            S = S16 * 16
                            if S == 0 or KSUB == "qi":
                                col0 += S16
                                continue
                            nch = S // 128
                            subs = []
                            for g0 in range(0, nch, GMAX):
                                gn = min(GMAX, nch - g0)
                                Ssub = gn * 128
                                ge = gep.tile([128, 3 * Ssub], BF, tag="ge",
                                              name=f"ge{r}_{x}_{g0}")
                                src_view = (h_all[0:SPLIT + 1, :] if x == 0
                                            else h_all[SPLIT:N, :])
                                nc.gpsimd.dma_gather(
                                    out_ap=ge.rearrange("p (k e) -> p k e", k=3)[:],
                                    in_ap=src_view,
                                    idxs_ap=cons["eidx"][:, col0 + g0 * 8:
                                                         col0 + g0 * 8 + Ssub // 16],
                                    num_idxs=Ssub, num_idxs_reg=Ssub,
                                    elem_size=F, transpose=True)
                                subs.append(ge.rearrange("p (k e) -> p k e", k=3))
                                if KSUB == "gath":
                                    nc.vector.tensor_reduce(
                                        out=sink[:, 0:1], in_=ge[:, 0:128],
                                        axis=mybir.AxisListType.X, op=ALU.max)
                                    nc.sync.dma_start(
                                        out=dbg_p[bass.ts(b, 128), 0:1],
                                        in_=sink[:, 0:1])
                            ges[(r, x)] = subs
                            col0 += S16
                    upb = []
                    for si in range(BS):
                        ut = pspool.tile([128, 400], F32, tag=f"u{si}", name=f"u{b}_{si}")
                        upb.append(ut)
                    started = [False] * BS
                    if KSUB in ("qi", "gath"):
                        # consume qis so it isn't DCE'd
                        nc.gpsimd.dma_start(out=dbg_p[bass.ts(b, 128), 4:4 + BS * R * 8],
                                          in_=qis[:])
                        continue
                    # last (r, x) group with chunks, per slot (to set stop=)
                    last_rx = {}
                    for si in range(BS):
                        for r in range(R):
                            for x in range(2):
                                if int(K[b * BS + si, r, x]) > 0:
                                    last_rx[si] = (r, x)
                    for r in range(R):
                        for x in range(2):
                            subs = ges.get((r, x))
                            cl = 0
                            for si in range(BS):
                                s = b * BS + si
                                for j in range(int(K[s, r, x])):
                                    gev = subs[cl // GMAX]
                                    off = (cl % GMAX) * 128
                                    cl += 1
                                    mp = mpool.tile([128, 256], BF, tag="mp")
                                    nc.sync.dma_start(out=mp[:],
                                                      in_=P["mstream"][ci, :, :])
                                    pt = pstmp.tile([128, 400], F32, tag="pt")
                                    for kk in range(3):
                                        nc.tensor.matmul(
                                            out=pt[:, 0:392],
                                            lhsT=gev[:, kk, off:off + 128],
                                            rhs=waugv[l][:, kk, r, :],
                                            start=(kk == 0), stop=(kk == 2))
                                    nc.tensor.matmul(
                                        out=pt[:, 392:400], lhsT=mp[:, 128:256],
                                        rhs=qisv[:, si, r, :], start=True, stop=True)
                                    qe = work.tile([128, 8], F32, tag="qe")
                                    nc.vector.tensor_copy(out=qe[:], in_=pt[:, 392:400])
                                    at = work.tile([128, 8], F32, tag="at")
                                    nc.vector.tensor_tensor(
                                        out=at[:], in0=pt[:, 384:392],
                                        in1=qe[:], op=ALU.add)
                                    at2 = work.tile([128, 8], F32, tag="at2")
                                    nc.vector.tensor_scalar_mul(at2[:], at[:], NEG)
                                    nc.vector.tensor_tensor(out=at[:], in0=at[:],
                                                            in1=at2[:], op=ALU.max)
                                    nc.scalar.activation(out=at[:], in_=at[:],
                                                         func=AF.Exp)
                                    me = work.tile([128, 392], BF, tag="me")
                                    nc.vector.tensor_tensor(
                                        out=me[:, 0:384].rearrange(
                                            "p (h c) -> p h c", h=H)[:],
                                        in0=pt[:, 0:384].rearrange(
                                            "p (h c) -> p h c", h=H)[:],
                                        in1=at[:].to_broadcast([128, H, C]),
                                        op=ALU.mult)
                                    nc.vector.tensor_copy(out=me[:, 384:392],
                                                          in_=at[:])
                                    is_last = (last_rx.get(si) == (r, x)
                                               and j == int(K[s, r, x]) - 1)
                                    nc.tensor.matmul(
                                        out=upb[si][:, 0:392], lhsT=mp[:, 0:128],
                                        rhs=me[:], start=not started[si],
                                        stop=is_last)
                                    started[si] = True
                                    ci += 1
                    if KSUB == "chunk":
                        for si in range(BS):
                            uo = work.tile([128, 400], F32, tag="uo")
                            nc.vector.tensor_copy(out=uo[:], in_=upb[si][:])
                            nc.sync.dma_start(out=dbg_p[bass.ts(b, 128), 0:384],
                                              in_=uo[:, 0:384])
                        continue
                    for si in range(BS):
                        s = b * BS + si
                        up = upb[si]
                        if not started[si]:
                            nc.vector.memset(up[:], 0.0)
                        sr = work.tile([128, 8], F32, tag="sr")
                        nc.vector.tensor_scalar_add(sr[:], up[:, 384:392], 1e-16)
                        nc.vector.reciprocal(out=sr[:], in_=sr[:])
                        z = work.tile([128, F], F32, tag="z")
                        nc.vector.tensor_tensor(
                            out=z.rearrange("p (h c) -> p h c", h=H)[:],
                            in0=up[:, 0:384].rearrange("p (h c) -> p h c", h=H)[:],
                            in1=sr[:].to_broadcast([128, H, C]), op=ALU.mult)
                        nc.vector.tensor_tensor(
                            out=z[:], in0=z[:],
                            in1=cons[f"rb{l}"][:],
                            op=ALU.add)
                        e1 = work.tile([128, F], F32, tag="e1")
                        nc.vector.tensor_scalar_min(e1[:], z[:], 0.0)
                        nc.scalar.activation(out=e1[:], in_=e1[:], func=AF.Exp)
                        nc.vector.tensor_scalar_add(e1[:], e1[:], -1.0)
                        nc.vector.tensor_tensor(out=z[:], in0=z[:], in1=e1[:],
                                                op=ALU.max)
                        nc.vector.tensor_tensor(out=z[:], in0=z[:],
                                                in1=hs[:, s, :], op=ALU.add)
                        nc.vector.tensor_copy(out=hs[:, s, :], in_=z[:])
                        rs = work.tile([128, 2], F32, tag="rs")
                        nc.vector.tensor_reduce(out=rs[:, 0:1], in_=z[:],
                                                axis=mybir.AxisListType.X,
                                                op=ALU.add)
                        sq2 = work.tile([128, F], F32, tag="sq")
                        nc.vector.tensor_tensor(out=sq2[:], in0=z[:], in1=z[:],
                                                op=ALU.mult)
                        nc.vector.tensor_reduce(out=rs[:, 1:2], in_=sq2[:],
                                                axis=mybir.AxisListType.X,
                                                op=ALU.add)
                        rsb = work.tile([128, 2], BF, tag="rsb")
                        nc.vector.tensor_copy(out=rsb[:], in_=rs[:])
                        bgt = work.tile([128, 8 * 128], BF, tag="bgt")
                        bgtv = bgt.rearrange("p (j g) -> p j g", j=8)
                        nc.sync.dma_start(
                            out=bgtv[:],
                            in_=P["bg"][s].rearrange("p (j g) -> p j g", j=8))
                        for jb in range(8):
                            nc.tensor.matmul(out=up[:, 16 * jb:16 * jb + 2],
                                             lhsT=bgtv[:, jb, :], rhs=rsb[:],
                                             start=True, stop=True)
                        gj = work.tile([128, 16], F32, tag="gj")
                        nc.vector.tensor_copy(
                            out=gj.rearrange("p (j e) -> p j e", j=8)[:],
                            in_=up[:, 0:128].rearrange(
                                "p (j e) -> p j e", j=8)[:, :, 0:2])
                        nc.vector.tensor_tensor(out=gslab[:], in0=gslab[:],
                                                in1=gj[:], op=ALU.add)
                if KSUB != "all":
                    continue
                nc.gpsimd.dma_start(out=g_b1[:], in_=gslab[:])
                nc.gpsimd.collective_compute("AllReduce", ALU.add,
                                             replica_groups=rg_all,
                                             ins=[g_b1.opt()], outs=[g_b2.opt()])
                gr = work.tile([128, 16], F32, tag="gr")
                nc.sync.dma_start(out=gr[:], in_=g_b2[:])
                grv = gr.rearrange("p (j e) -> p j e", j=8)
                mean = work.tile([128, 8], F32, tag="mean")
                nc.vector.tensor_tensor(out=mean[:], in0=grv[:, :, 0],
                                        in1=cons["rcnt"][:], op=ALU.mult)
                ex2 = work.tile([128, 8], F32, tag="ex2")
                nc.vector.tensor_tensor(out=ex2[:], in0=grv[:, :, 1],
                                        in1=cons["rcnt"][:], op=ALU.mult)
                msq = work.tile([128, 8], F32, tag="msq")
                nc.vector.tensor_tensor(out=msq[:], in0=mean[:], in1=mean[:],
                                        op=ALU.mult)
                nc.vector.tensor_tensor(out=ex2[:], in0=ex2[:], in1=msq[:],
                                        op=ALU.subtract)
                nc.vector.tensor_scalar_add(ex2[:], ex2[:], EPS)
                nc.scalar.activation(out=ex2[:], in_=ex2[:], func=AF.Sqrt)
                nc.vector.reciprocal(out=ex2[:], in_=ex2[:])
                stw = work.tile([128, 16], F32, tag="stw")
                stwv = stw.rearrange("p (j e) -> p j e", j=8)
                nc.vector.tensor_copy(out=stwv[:, :, 0], in_=mean[:])
                nc.vector.tensor_copy(out=stwv[:, :, 1], in_=ex2[:])
                nc.sync.dma_start(
                    out=stats_t.rearrange("(j p) e -> p j e", p=128)[:, :, 0:2],
                    in_=stwv[:])
                stg = sgp.tile([128, NSLOT * 64], F32, tag="stg")
                stgv = stg.rearrange("p (s e) -> p s e", s=NSLOT)
                for gb in range(NBATCH):
                    nc.gpsimd.dma_gather(
                        out_ap=stgv[:, gb * BS:(gb + 1) * BS, :],
                        in_ap=stats_t[:],
                        idxs_ap=cons["gidx"][:, gb * BS * 8:(gb + 1) * BS * 8],
                        num_idxs=BS * 128, num_idxs_reg=BS * 128,
                        elem_size=64, transpose=False)
                for s in range(NSLOT):
                    t = work.tile([128, F], F32, tag="z")
                    nc.vector.tensor_scalar(
                        out=t[:], in0=hs[:, s, :],
                        scalar1=stgv[:, s, 0:1], scalar2=stgv[:, s, 1:2],
                        op0=ALU.subtract, op1=ALU.mult)
                    nc.vector.tensor_tensor(
                        out=t[:], in0=t[:],
                        in1=cons[f"nw{l}"][:],
                        op=ALU.mult)
                    nc.vector.tensor_tensor(
                        out=t[:], in0=t[:],
                        in1=cons[f"nb{l}"][:],
                        op=ALU.add)
                    nc.vector.tensor_copy(out=hs[:, s, :], in_=t[:])
                    nc.sync.dma_start(out=h_local[bass.ts(s, 128), :],
                                      in_=hs[:, s, :])

            if STAGE == "l1":
                for s in range(NSLOT):
                    t = work.tile([128, F], F32, tag="hb")
                    nc.vector.tensor_copy(out=t[:], in_=hs[:, s, :])
                    nc.sync.dma_start(out=dbg_p[bass.ts(s, 128), :], in_=t[:])
            # ================= head =================
            for b in range(NBATCH if STAGE == "full" else 0):
                hts = htp.tile([128, 3 * BS * 128], BF, tag="hts")
                htsv = hts.rearrange("p (k e) -> p k e", k=3)
                nc.gpsimd.dma_gather(
                    out_ap=htsv[:], in_ap=h_local[:],
                    idxs_ap=cons["sidx"][:, b * BS * 8:(b + 1) * BS * 8],
                    num_idxs=BS * 128, num_idxs_reg=BS * 128,
                    elem_size=F, transpose=True)
                for si in range(BS):
                    s = b * BS + si
                    pz = pstmp.tile([128, 400], F32, tag="pt")
                    for kk in range(3):
                        nc.tensor.matmul(out=pz[:, 0:128], lhsT=cw1v[:, kk, :],
                                         rhs=htsv[:, kk, bass.ts(si, 128)],
                                         start=(kk == 0), stop=(kk == 2))
                    z1 = work.tile([128, 128], BF, tag="z1")
                    nc.scalar.activation(out=z1[:], in_=pz[:, 0:128],
                                         func=AF.Relu, bias=cons["cb1"][:])
                    po = pstmp.tile([128, 400], F32, tag="pt")
                    nc.tensor.matmul(out=po[:, 0:49], lhsT=z1[:],
                                     rhs=cons["cw2"][:], start=True, stop=True)
                    ot = work.tile([128, 49], F32, tag="ot")
                    nc.vector.tensor_tensor(
                        out=ot[:], in0=po[:, 0:49],
                        in1=cons["cb2"][:],
                        op=ALU.add)
                    nc.sync.dma_start(out=out_p[bass.ts(s, 128), :], in_=ot[:])

    nc.compile()
    return nc


def kernel(**inputs):
    pp = preprocess(inputs)
    nc = build_kernel(pp)
    in_maps = []
    for c in range(NCORES):
        m = dict(pp["shared"])
        m.update(pp["percore"][c])
        in_maps.append(m)
    res = run_bass_kernel_spmd(nc, in_maps, core_ids=list(range(NCORES)))
    out = np.concatenate([res.results[c]["out"] for c in range(NCORES)], axis=0)
    return out.astype(np.float32)


if __name__ == "__main__":
    import time
    import jax
    import reference
    t0 = time.perf_counter()
    with jax.default_device(jax.devices("cpu")[0]):
        inputs = {k: np.asarray(v) for k, v in reference.setup_inputs().items()}
        exp = np.asarray(reference.reference(**inputs))
    print(f"reference done in {time.perf_counter()-t0:.1f}s")
    t0 = time.perf_counter()
    got = kernel(**inputs)
    print(f"kernel done in {time.perf_counter()-t0:.1f}s")
    rel = np.linalg.norm(got - exp) / (np.linalg.norm(exp) + 1e-30)
    mx = np.abs(got - exp).max()
    print(f"Relative error: {rel:.4e}   max-abs: {mx:.3e}  exp-scale: {np.abs(exp).max():.3f}")
